# revision 14
# baseline (speedup 1.0000x reference)
"""Trainium2 Bass kernel for ChannelSelection (top-k channel masking).

Reference computation (per vehicle n of N=4):
  s = 0.5*grad_mag(x) + 0.5*|x|            # grad_mag = |x[w+1]-x[w-1]| + |x[h+1]-x[h-1]|
  sp[c, patch] = mean of s over 32x32 patch
  keep top-128 (of 256) channels per patch (rank by sp desc, stable)
  out = x * mask broadcast over patch

Sharding: 8 cores = N(4) x H-halves(2). Each core gets one vehicle's
channel-complete slab of 128 rows (+1 halo row each side, zero padded)
and computes its patches' top-k independently (patches never straddle
the H split since 128 % 32 == 0).

Device algorithm per core (channels on partitions, 2 groups of 128):
  per 32-row strip (4 strips):
    DMA in x tile (128, 34, 258)  [1px zero halo baked in by host]
    DVE : ex = x[w+1]-x[w-1], ey = x[h+1]-x[h-1]  (tensor_tensor sub)
    ACT : Abs activation with accum_out -> per-patch sum of |ex|+|ey|
    DVE/ACT: |x| patch sums via tensor_scalar(abs_max, 0) accum / Abs accum
    sp = sum|ex|+|ey| + sum|x|   (scale-free: ranks invariant to pos. scaling)
    PE  : transpose sp (128,8) -> spT, broadcast each patch row to 128
          partitions via ones-matmul
    ACT : Sign(spT_bcast - sp) with accum -> sgn[c] = #gt - #lt
    keep channel c iff sgn[c] <= -1  (i.e. fewer than 128 channels beat it)
    DVE : x *= mask (per-partition scalar multiply), in place
    DMA out (128, 32, 256)
"""

import os
import sys

import numpy as np

_TRN_REPO = "/opt/trn_rl_repo"
if _TRN_REPO not in sys.path:
    sys.path.insert(0, _TRN_REPO)

# Full-problem constants (hardcoded per contest rules)
N_VEH = 4
C = 256
H = 256
W = 256
P = 32          # patch size
N_CORES = 8
HS = 128        # rows per core (H/2)

_cache = {}


def build_program(ns=4, npc=8, xsplit=2):
    """Build the SPMD Bass program for one core.

    ns: number of 32-row strips (ns*32 = rows per core)
    npc: number of patch columns (npc*32 = W)
    xsplit: |x|-absacc patches per strip assigned to DVE (rest go to ACT)
    """
    from contextlib import ExitStack

    import concourse.bass as bass
    import concourse.tile as tile
    from concourse import bacc, masks, mybir

    f32 = mybir.dt.float32
    Alu = mybir.AluOpType
    Act = mybir.ActivationFunctionType

    rows = ns * P
    w = npc * P
    wp = w + 2

    nc = bacc.Bacc("TRN2", target_bir_lowering=False, debug=False)
    x_ap = nc.dram_tensor("x", [C, rows + 2, wp], f32, kind="ExternalInput").ap()
    oh_ap = nc.dram_tensor(
        "onehot", [npc, npc * 128], f32, kind="ExternalInput"
    ).ap()
    o_ap = nc.dram_tensor("out", [C, rows, w], f32, kind="ExternalOutput").ap()

    NQ = npc // 2  # quarters per strip-group (64 cols each)

    with tile.TileContext(nc) as tc, ExitStack() as ctx:
        const_pool = ctx.enter_context(tc.tile_pool(name="const", bufs=1))
        x_pool = ctx.enter_context(tc.tile_pool(name="xs", bufs=4))
        exy_pool = ctx.enter_context(tc.tile_pool(name="exy", bufs=2))
        acc_pool = ctx.enter_context(tc.tile_pool(name="acc", bufs=4))
        spt_pool = ctx.enter_context(tc.tile_pool(name="spt", bufs=2))
        scr_pool = ctx.enter_context(tc.tile_pool(name="scr", bufs=1))
        ps_exy = ctx.enter_context(tc.tile_pool(name="ps_exy", bufs=1, space="PSUM"))
        ps_b = ctx.enter_context(tc.tile_pool(name="ps_b", bufs=1, space="PSUM"))
        ps_sgn = ctx.enter_context(tc.tile_pool(name="ps_sgn", bufs=1, space="PSUM"))
        ps_t = ctx.enter_context(tc.tile_pool(name="ps_t", bufs=2, space="PSUM"))

        # constants
        ident = const_pool.tile([128, 128], f32)
        masks.make_identity(nc, ident[:])
        # onehot[p, 128p:128(p+1)] = 1: stationary for the patch-row broadcast
        onehot = const_pool.tile([npc, npc * 128], f32)
        nc.sync.dma_start(onehot[:], oh_ap[:])

        # engine-private scratch (activation mandatory outputs)
        act_dump = scr_pool.tile([128, P, P], f32, tag="act_dump")
        psum_dump = ps_exy.tile([128, 2, P, P], f32, tag="psum_dump")

        for s in range(ns):
            r0 = s * P
            xt = []
            for g in range(2):
                t = x_pool.tile([128, P + 2, wp], f32, tag="x")
                nc.sync.dma_start(
                    t[:], x_ap[g * 128:(g + 1) * 128, r0:r0 + P + 2, :]
                )
                xt.append(t)

            spA = []   # ACT |ex|+|ey| patch sums
            spXd = []  # DVE |x| patch sums
            spXa = []  # ACT |x| patch sums
            for g in range(2):
                a = acc_pool.tile([128, npc], f32, tag="spA")
                xd = acc_pool.tile([128, npc], f32, tag="spXd")
                xa = acc_pool.tile([128, npc], f32, tag="spXa")
                spA.append(a)
                spXd.append(xd)
                spXa.append(xa)

                for q in range(NQ):
                    c0 = 64 * q
                    et = exy_pool.tile([128, 2, P, 64], f32, tag="exy")
                    # ex = x[., w+1] - x[., w-1] over output cols [c0, c0+64)
                    nc.vector.tensor_tensor(
                        et[:, 0, :, :],
                        xt[g][:, 1:P + 1, c0 + 2:c0 + 66],
                        xt[g][:, 1:P + 1, c0:c0 + 64],
                        op=Alu.subtract,
                    )
                    # ey = x[h+1, .] - x[h-1, .]
                    nc.vector.tensor_tensor(
                        et[:, 1, :, :],
                        xt[g][:, 2:P + 2, c0 + 1:c0 + 65],
                        xt[g][:, 0:P, c0 + 1:c0 + 65],
                        op=Alu.subtract,
                    )
                    for hh in range(2):
                        p = 2 * q + hh
                        # sum |ex| + |ey| over patch p (ScalarE, fused abs+acc)
                        nc.scalar.activation(
                            psum_dump[:],
                            et[:, :, :, 32 * hh:32 * hh + 32],
                            Act.Abs,
                            accum_out=a[:, p:p + 1],
                        )

                # |x| patch sums, split DVE/ACT
                for p in range(npc):
                    xin = xt[g][:, 1:P + 1, 1 + P * p:1 + P * (p + 1)]
                    if p < xsplit:
                        nc.vector.tensor_reduce(
                            xd[:, p:p + 1], xin,
                            axis=mybir.AxisListType.XY, op=Alu.add,
                            apply_absolute_value=True,
                        )
                    else:
                        nc.scalar.activation(
                            act_dump[:], xin, Act.Abs,
                            accum_out=xa[:, p:p + 1],
                        )

            # finalize sp and negate for the Sign bias
            sp = []
            nsp = []
            for g in range(2):
                spg = acc_pool.tile([128, npc], f32, tag="sp")
                if xsplit > 0:
                    nc.vector.tensor_tensor(
                        spg[:, 0:xsplit], spA[g][:, 0:xsplit],
                        spXd[g][:, 0:xsplit], op=Alu.add,
                    )
                if xsplit < npc:
                    nc.vector.tensor_tensor(
                        spg[:, xsplit:npc], spA[g][:, xsplit:npc],
                        spXa[g][:, xsplit:npc], op=Alu.add,
                    )
                nspg = acc_pool.tile([128, npc], f32, tag="nsp")
                nc.vector.tensor_scalar(
                    nspg[:], spg[:], -1.0, None, op0=Alu.mult
                )
                sp.append(spg)
                nsp.append(nspg)

            # transpose sp -> spT (npc, 256): patch rows, channel cols
            spT = spt_pool.tile([npc, 256], f32, tag="spT")
            for g in range(2):
                pt = ps_t.tile([npc, 128], f32, tag="psT")
                nc.tensor.transpose(pt[:], sp[g][:], ident[:])
                nc.vector.tensor_copy(spT[:, g * 128:(g + 1) * 128], pt[:])

            # per patch: broadcast spT row to 128 partitions, Sign-count
            sgn = [
                acc_pool.tile([128, npc], f32, tag="sgn", name=f"sgn{g}")
                for g in range(2)
            ]
            for p in range(npc):
                pb = ps_b.tile([128, 256], f32, tag="pb")
                nc.tensor.matmul(
                    pb[:], onehot[:, 128 * p:128 * (p + 1)], spT[:],
                    start=True, stop=True,
                )
                for g in range(2):
                    po = ps_sgn.tile([128, 256], f32, tag="po")
                    nc.scalar.activation(
                        po[:], pb[:], Act.Sign,
                        bias=nsp[g][:, p:p + 1],
                        accum_out=sgn[g][:, p:p + 1],
                    )

            # mask = (sgn <= -1): fewer than k channels strictly beat c
            mask = []
            for g in range(2):
                mg = acc_pool.tile([128, npc], f32, tag="mask")
                nc.vector.tensor_scalar(
                    mg[:], sgn[g][:], -0.5, None, op0=Alu.is_le
                )
                mask.append(mg)

            # apply mask in place and store
            for g in range(2):
                for p in range(npc):
                    reg = xt[g][:, 1:P + 1, 1 + P * p:1 + P * (p + 1)]
                    nc.vector.tensor_scalar(
                        reg, reg, mask[g][:, p:p + 1], None, op0=Alu.mult
                    )
                nc.sync.dma_start(
                    o_ap[g * 128:(g + 1) * 128, r0:r0 + P, :],
                    xt[g][:, 1:P + 1, 1:w + 1],
                )

    nc.compile()
    return nc


def onehot_input(npc=8):
    oh = np.zeros((npc, npc * 128), np.float32)
    for p in range(npc):
        oh[p, 128 * p:128 * (p + 1)] = 1.0
    return oh


def _get_program():
    key = "full"
    if key not in _cache:
        _cache[key] = build_program()
    return _cache[key]


def kernel(x):
    """x: (4, 256, 256, 256) float32 -> masked output, same shape."""
    from concourse.bass_utils import run_bass_kernel_spmd

    x = np.asarray(x)
    assert x.shape == (N_VEH, C, H, W) and x.dtype == np.float32

    nc = _get_program()

    xp = np.pad(x, ((0, 0), (0, 0), (1, 1), (1, 1)))
    oh = onehot_input()
    in_maps = []
    for n in range(N_VEH):
        for hh in range(2):
            shard = np.ascontiguousarray(xp[n, :, hh * HS:hh * HS + HS + 2, :])
            in_maps.append({"x": shard, "onehot": oh})

    res = run_bass_kernel_spmd(nc, in_maps, list(range(N_CORES)))

    out = np.empty((N_VEH, C, H, W), np.float32)
    for n in range(N_VEH):
        for hh in range(2):
            out[n, :, hh * HS:hh * HS + HS, :] = res.results[n * 2 + hh]["out"]
    return out


# revision 18
# speedup vs baseline: 20.9645x; 20.9645x over previous
"""Trainium2 Bass kernel for ChannelSelection (top-k channel masking).

Reference computation (per vehicle n of N=4):
  s = 0.5*grad_mag(x) + 0.5*|x|            # grad_mag = |x[w+1]-x[w-1]| + |x[h+1]-x[h-1]|
  sp[c, patch] = mean of s over 32x32 patch
  keep top-128 (of 256) channels per patch (rank by sp desc, stable)
  out = x * mask broadcast over patch

Sharding: 8 cores = N(4) x H-halves(2). Each core gets one vehicle's
channel-complete slab of 128 rows (+1 halo row each side, zero padded)
and computes its patches' top-k independently (patches never straddle
the H split since 128 % 32 == 0).

Device algorithm per core (channels on partitions, 2 groups of 128):
  per 32-row strip (4 strips):
    DMA in x tile (128, 34, 258)  [1px zero halo baked in by host]
    DVE : ex = x[w+1]-x[w-1], ey = x[h+1]-x[h-1]  (tensor_tensor sub)
    ACT : Abs activation with accum_out -> per-patch sum of |ex|+|ey|
    DVE/ACT: |x| patch sums via tensor_scalar(abs_max, 0) accum / Abs accum
    sp = sum|ex|+|ey| + sum|x|   (scale-free: ranks invariant to pos. scaling)
    PE  : transpose sp (128,8) -> spT, broadcast each patch row to 128
          partitions via ones-matmul
    ACT : Sign(spT_bcast - sp) with accum -> sgn[c] = #gt - #lt
    keep channel c iff sgn[c] <= -1  (i.e. fewer than 128 channels beat it)
    DVE : x *= mask (per-partition scalar multiply), in place
    DMA out (128, 32, 256)
"""

import os
import sys

import numpy as np

_TRN_REPO = "/opt/trn_rl_repo"
if _TRN_REPO not in sys.path:
    sys.path.insert(0, _TRN_REPO)

# Full-problem constants (hardcoded per contest rules)
N_VEH = 4
C = 256
H = 256
W = 256
P = 32          # patch size
N_CORES = 8
HS = 128        # rows per core (H/2)

_cache = {}


def build_program(ns=4, npc=8, xsplit=2, reps=1):
    """Build the SPMD Bass program for one core.

    ns: number of 32-row strips (ns*32 = rows per core)
    npc: number of patch columns (npc*32 = W)
    xsplit: |x|-absacc patches per strip assigned to DVE (rest go to ACT)
    reps: repeat the whole pipeline (timing harness only)
    """
    from contextlib import ExitStack

    import concourse.bass as bass
    import concourse.tile as tile
    from concourse import bacc, masks, mybir

    f32 = mybir.dt.float32
    Alu = mybir.AluOpType
    Act = mybir.ActivationFunctionType

    rows = ns * P
    w = npc * P
    wp = w + 2

    nc = bacc.Bacc("TRN2", target_bir_lowering=False, debug=False)
    x_ap = nc.dram_tensor("x", [C, rows + 2, wp], f32, kind="ExternalInput").ap()
    oh_ap = nc.dram_tensor(
        "onehot", [npc, npc * 128], f32, kind="ExternalInput"
    ).ap()
    o_ap = nc.dram_tensor("out", [C, rows, w], f32, kind="ExternalOutput").ap()

    NQ = npc // 2  # quarters per strip-group (64 cols each)

    with tile.TileContext(nc) as tc, ExitStack() as ctx:
        const_pool = ctx.enter_context(tc.tile_pool(name="const", bufs=1))
        x_pool = ctx.enter_context(tc.tile_pool(name="xs", bufs=4))
        exy_pool = ctx.enter_context(tc.tile_pool(name="exy", bufs=2))
        acc_pool = ctx.enter_context(tc.tile_pool(name="acc", bufs=4))
        spt_pool = ctx.enter_context(tc.tile_pool(name="spt", bufs=2))
        scr_pool = ctx.enter_context(tc.tile_pool(name="scr", bufs=1))
        ps_exy = ctx.enter_context(tc.tile_pool(name="ps_exy", bufs=1, space="PSUM"))
        ps_b = ctx.enter_context(tc.tile_pool(name="ps_b", bufs=1, space="PSUM"))
        ps_sgn = ctx.enter_context(tc.tile_pool(name="ps_sgn", bufs=1, space="PSUM"))
        ps_t = ctx.enter_context(tc.tile_pool(name="ps_t", bufs=2, space="PSUM"))

        # constants
        ident = const_pool.tile([128, 128], f32)
        masks.make_identity(nc, ident[:])
        # onehot[p, 128p:128(p+1)] = 1: stationary for the patch-row broadcast
        onehot = const_pool.tile([npc, npc * 128], f32)
        nc.sync.dma_start(onehot[:], oh_ap[:])

        # engine-private scratch (activation mandatory outputs)
        act_dump = scr_pool.tile([128, P, P], f32, tag="act_dump")
        psum_dump = ps_exy.tile([128, 2, P, P], f32, tag="psum_dump")

        for rep in range(reps):
            _run_strips(nc, tc, ctx, locals())

    nc.compile()
    return nc


def _run_strips(nc, tc, ctx, env):
    import concourse.bass as bass
    from concourse import mybir

    f32 = mybir.dt.float32
    Alu = mybir.AluOpType
    Act = mybir.ActivationFunctionType
    ns, npc, xsplit = env["ns"], env["npc"], env["xsplit"]
    w, wp = env["w"], env["wp"]
    x_ap, o_ap = env["x_ap"], env["o_ap"]
    x_pool, exy_pool, acc_pool = env["x_pool"], env["exy_pool"], env["acc_pool"]
    spt_pool = env["spt_pool"]
    ps_b, ps_sgn, ps_t = env["ps_b"], env["ps_sgn"], env["ps_t"]
    ident, onehot = env["ident"], env["onehot"]
    act_dump, psum_dump = env["act_dump"], env["psum_dump"]
    NQ = env["NQ"]

    if True:
        for s in range(ns):
            r0 = s * P
            xt = []
            for g in range(2):
                t = x_pool.tile([128, P + 2, wp], f32, tag="x")
                nc.sync.dma_start(
                    t[:], x_ap[g * 128:(g + 1) * 128, r0:r0 + P + 2, :]
                )
                xt.append(t)

            spA = []   # ACT |ex|+|ey| patch sums
            spXd = []  # DVE |x| patch sums
            spXa = []  # ACT |x| patch sums
            for g in range(2):
                a = acc_pool.tile([128, npc], f32, tag="spA")
                xd = acc_pool.tile([128, npc], f32, tag="spXd")
                xa = acc_pool.tile([128, npc], f32, tag="spXa")
                spA.append(a)
                spXd.append(xd)
                spXa.append(xa)

                for q in range(NQ):
                    c0 = 64 * q
                    et = exy_pool.tile([128, 2, P, 64], f32, tag="exy")
                    # ex = x[., w+1] - x[., w-1] over output cols [c0, c0+64)
                    nc.vector.tensor_tensor(
                        et[:, 0, :, :],
                        xt[g][:, 1:P + 1, c0 + 2:c0 + 66],
                        xt[g][:, 1:P + 1, c0:c0 + 64],
                        op=Alu.subtract,
                    )
                    # ey = x[h+1, .] - x[h-1, .]
                    nc.vector.tensor_tensor(
                        et[:, 1, :, :],
                        xt[g][:, 2:P + 2, c0 + 1:c0 + 65],
                        xt[g][:, 0:P, c0 + 1:c0 + 65],
                        op=Alu.subtract,
                    )
                    for hh in range(2):
                        p = 2 * q + hh
                        # sum |ex| + |ey| over patch p (ScalarE, fused abs+acc)
                        nc.scalar.activation(
                            psum_dump[:],
                            et[:, :, :, 32 * hh:32 * hh + 32],
                            Act.Abs,
                            accum_out=a[:, p:p + 1],
                        )

                # |x| patch sums, split DVE/ACT
                for p in range(npc):
                    xin = xt[g][:, 1:P + 1, 1 + P * p:1 + P * (p + 1)]
                    if p < xsplit:
                        nc.vector.tensor_reduce(
                            xd[:, p:p + 1], xin,
                            axis=mybir.AxisListType.XY, op=Alu.add,
                            apply_absolute_value=True,
                        )
                    else:
                        nc.scalar.activation(
                            act_dump[:], xin, Act.Abs,
                            accum_out=xa[:, p:p + 1],
                        )

            # finalize sp and negate for the Sign bias
            sp = []
            nsp = []
            for g in range(2):
                spg = acc_pool.tile([128, npc], f32, tag="sp")
                if xsplit > 0:
                    nc.vector.tensor_tensor(
                        spg[:, 0:xsplit], spA[g][:, 0:xsplit],
                        spXd[g][:, 0:xsplit], op=Alu.add,
                    )
                if xsplit < npc:
                    nc.vector.tensor_tensor(
                        spg[:, xsplit:npc], spA[g][:, xsplit:npc],
                        spXa[g][:, xsplit:npc], op=Alu.add,
                    )
                nspg = acc_pool.tile([128, npc], f32, tag="nsp")
                nc.vector.tensor_scalar(
                    nspg[:], spg[:], -1.0, None, op0=Alu.mult
                )
                sp.append(spg)
                nsp.append(nspg)

            # transpose sp -> spT (npc, 256): patch rows, channel cols
            spT = spt_pool.tile([npc, 256], f32, tag="spT")
            for g in range(2):
                pt = ps_t.tile([npc, 128], f32, tag="psT")
                nc.tensor.transpose(pt[:], sp[g][:], ident[:])
                nc.vector.tensor_copy(spT[:, g * 128:(g + 1) * 128], pt[:])

            # per patch: broadcast spT row to 128 partitions, Sign-count
            sgn = [
                acc_pool.tile([128, npc], f32, tag="sgn", name=f"sgn{g}")
                for g in range(2)
            ]
            for p in range(npc):
                pb = ps_b.tile([128, 256], f32, tag="pb")
                nc.tensor.matmul(
                    pb[:], onehot[:, 128 * p:128 * (p + 1)], spT[:],
                    start=True, stop=True,
                )
                for g in range(2):
                    po = ps_sgn.tile([128, 256], f32, tag="po")
                    nc.scalar.activation(
                        po[:], pb[:], Act.Sign,
                        bias=nsp[g][:, p:p + 1],
                        accum_out=sgn[g][:, p:p + 1],
                    )

            # mask = (sgn <= -1): fewer than k channels strictly beat c
            mask = []
            for g in range(2):
                mg = acc_pool.tile([128, npc], f32, tag="mask")
                nc.vector.tensor_scalar(
                    mg[:], sgn[g][:], -0.5, None, op0=Alu.is_le
                )
                mask.append(mg)

            # apply mask in place and store
            for g in range(2):
                for p in range(npc):
                    reg = xt[g][:, 1:P + 1, 1 + P * p:1 + P * (p + 1)]
                    nc.vector.tensor_scalar(
                        reg, reg, mask[g][:, p:p + 1], None, op0=Alu.mult
                    )
                nc.sync.dma_start(
                    o_ap[g * 128:(g + 1) * 128, r0:r0 + P, :],
                    xt[g][:, 1:P + 1, 1:w + 1],
                )

    nc.compile()
    return nc


def onehot_input(npc=8):
    oh = np.zeros((npc, npc * 128), np.float32)
    for p in range(npc):
        oh[p, 128 * p:128 * (p + 1)] = 1.0
    return oh


def _get_program():
    key = "full"
    if key not in _cache:
        _cache[key] = build_program()
    return _cache[key]


def kernel(x):
    """x: (4, 256, 256, 256) float32 -> masked output, same shape."""
    from concourse.bass_utils import run_bass_kernel_spmd

    x = np.asarray(x)
    assert x.shape == (N_VEH, C, H, W) and x.dtype == np.float32

    nc = _get_program()

    xp = np.pad(x, ((0, 0), (0, 0), (1, 1), (1, 1)))
    oh = onehot_input()
    in_maps = []
    for n in range(N_VEH):
        for hh in range(2):
            shard = np.ascontiguousarray(xp[n, :, hh * HS:hh * HS + HS + 2, :])
            in_maps.append({"x": shard, "onehot": oh})

    res = run_bass_kernel_spmd(nc, in_maps, list(range(N_CORES)))

    out = np.empty((N_VEH, C, H, W), np.float32)
    for n in range(N_VEH):
        for hh in range(2):
            out[n, :, hh * HS:hh * HS + HS, :] = res.results[n * 2 + hh]["out"]
    return out
